# revision 32
# baseline (speedup 1.0000x reference)
"""Trainium2 Bass kernel for nn_ContextProjector (moe_routing).

Reference computation:
    projected = split_heads(x @ W_x + b_x)            # (B,H,N,D)
    fx        = split_heads(x @ W_fx + b_fx)          # (B,H,N,D)
    sp        = projected @ W_slice + b_slice         # (B,H,N,S)
    w         = softmax(sp / clip(temp,.5,5))         # (B,H,N,S)
    norm      = w.sum(axis=N)                         # (B,H,S)
    out       = einsum('bhns,bhnd->bhsd', w/(norm+.01), fx)

Key algebraic restructuring (all exact):
  * projected is only used for sp, so fold on host:
        Wc[c,(h,s)] = sum_d W_x[c,(h,d)] W_slice[d,s] / t[h]
    and sp/t = x @ Wc + bc.  The additive bias bc is applied two ways,
    balancing PE against DVE: for NB pairs per oct via the classic
    ones-row matmul into PSUM (PE), for the rest multiplicatively after
    exp (exp(lg+bc) = exp(lg)*E) as a coalesced 2x-mode DVE multiply.
  * fx never exists on device. With w~ the per-token softmax:
        sum_n w~[n,s] (x[n,:] @ W_fx + b_fx)[d]
          = (sum_n w~[n,s] [x[n,:] | 1]) @ [W_fx; b_fx]
    so the device only accumulates G[(h,s), c] = sum_n w~[n,(h,s)] [x|1][n,c]
    into PSUM; the tiny G @ W_fx, the b_fx term, and the final divide by
    (norm+0.01) happen on host in float64. Column c=C of G is the norm.

Column layout is S-MAJOR (col = s*H + h): the softmax denominator then
comes from a log2(S)-deep halving tree of CONTIGUOUS-slice adds (DVE runs
at ~0.5 ns/elem on packed runs vs ~2.3 ns/elem for strided reduces, and
~13 ns per access-pattern row on broadcasts).

All matmul operands are BFLOAT16 (PSUM accumulates fp32): bf16 streams at
the full 2.4 GHz warm PE clock (one rhs column per cycle), whereas fp16
measured ~20% slower; bf16 also converts to the DVE's internal fp32 for
free.  Measured per-op rates (fast clock): N=512 matmul issue gap 216 ns,
N=257 gap 110 ns (LDWEIGHTS fully hidden), DVE TT bf16 2x-mode
0.52 ns/elem + ~162 ns/op, ACT 1x dtype-independent (FD+352)/1.2 ns.

Engine assignment per OCT (8 subtiles = 1024 tokens; fixed costs amortized):
  PE : per pair: lg2 psum = 2 K-chunk matmuls per subtile (+ bias matmul
       for the NBS=1 last subtile); per oct: 32 G matmuls (4 accs x 8
       subtiles), ap 257.  PE is the bottleneck engine (~93% occupancy,
       near the 78.6 TF/s roofline for the 8.6 GF of work per core).
  ACT: 4x exp [128,2,512] PSUM->SBUF per oct; recH = half-width broadcast
       expansion of rec, Copy activation
  DVE: in-place E-multiply on the non-bias subtiles (exp(lg+bc)=exp(lg)*E);
       halving tree to den (bf16, last level f32);
       reciprocal_approx_fast (f32, ~5x cheaper than reciprocal);
       2x coalesced normalize (two halves vs recH)
  GpS: only head/tail DMA descriptor generation (its compute ops contend
       for the DVE's shared SBUF port)
The recH expansion and normalize for oct q are deferred to oct q+2's
emission slot, which breaks the exp->E->tree->recip->recH->exp serial
cycle across octs.  G matmuls run PD=4 octs behind their producers (the
deep pending queue keeps the PE busy over the tail finish-chains).
Ramp/tail PE idle is filled with dependency-free matmuls into the
not-yet-started G accumulators (the first real G matmul clears them via
start=True).  Constants bc/e2 are built on device from a single [1,HS]
row (K=1 ones matmul broadcast + exp) instead of replicated DMAs; the
head x/wc DMAs are split so first compute starts ~2.5 us after the NEFF
preamble; the final oct emits G j-major so the 4 accumulator stores
pipeline; store DMA queues are pre-warmed.
"""

import numpy as np
import ml_dtypes

import concourse.bass as bass
import concourse.mybir as mybir
import concourse.tile as tile
from concourse import bacc
from concourse.bass_utils import run_bass_kernel_spmd

# Problem shape (hardcoded per contract)
B, N, C = 2, 65536, 256
H, D, S = 8, 64, 64
HS = H * S    # 512
P = 128
NCORES = 8
SHARDS_PER_B = NCORES // B   # 4
T = N // SHARDS_PER_B        # 16384 tokens per core
CA = C + 1                   # token-major x augmented with a ones column

PD = 4     # G-matmul pipeline depth (octs)
TT = 4096  # tokens per DMA block
QT = 1024  # tokens per oct (chain work unit)
QS = QT // P                 # 8 subtiles per oct
NBS = 1    # subtiles per oct whose bias rides the PE (rest use DVE E-mult)
NES = QS - NBS               # subtiles per oct using the DVE E-multiply

f16 = mybir.dt.bfloat16
f32 = mybir.dt.float32


def _emit(ctx, tc, xt, wc, bcr, xtm, out, t_tokens):
    nc = tc.nc
    KO = C // P              # 2 K-chunks of x
    n_blk = t_tokens // TT
    n_sub = TT // P          # subtiles (128 tokens) per block
    n_tot = t_tokens // P    # total subtiles (for G start/stop flags)

    consts = ctx.enter_context(tc.tile_pool(name="consts", bufs=1))
    xpool = ctx.enter_context(tc.tile_pool(name="xpool", bufs=2))
    mpool = ctx.enter_context(tc.tile_pool(name="mpool", bufs=3))
    wpool = ctx.enter_context(tc.tile_pool(name="wpool", bufs=PD + 4))
    mpool_extra = None  # xm lifetime covered by mpool bufs below
    vpool = ctx.enter_context(tc.tile_pool(name="vpool", bufs=3))
    spool = ctx.enter_context(tc.tile_pool(name="spool", bufs=4))
    ppool = ctx.enter_context(tc.tile_pool(name="ppool", bufs=2, space="PSUM"))
    apool = ctx.enter_context(tc.tile_pool(name="apool", bufs=1, space="PSUM"))
    opool = ctx.enter_context(tc.tile_pool(name="opool", bufs=1))

    # Constant weights, resident in SBUF for the whole kernel.  Split the
    # wc DMA per K-chunk so the first logits matmul only waits on chunk 0.
    wc_r = wc[:].rearrange("(ko ki) n -> ki ko n", ki=P)
    wc_sb = consts.tile([P, KO, HS], f16)
    nc.sync.dma_start(wc_sb[:, 0, :], wc_r[:, 0, :])
    # bcr: only row 0 (= bc) is real; rows 1-127 are zeroed so the
    # ones-row bias matmul multiplies them by zero safely.
    bcr_sb = consts.tile([P, HS], f16)
    nc.vector.memset(bcr_sb[:], 0.0)
    # e2 = exp(bc) broadcast to all partitions, built on device: a K=1
    # ones-column matmul replicates bc across partitions, then one exp.
    e2_sb = consts.tile([P, HS], f16)
    # Bias K-chunk lhsT: row 0 ones, rest zero -> adds bcr row 0 once.
    xpad = consts.tile([P, P], f16)
    nc.vector.memset(xpad[:], 0.0)
    nc.vector.memset(xpad[0:1, :], 1.0)

    # Persistent PSUM accumulators: chunk j holds
    # G[cols 128j..128j+128, 257] = sum_n w~[n, col] * [x[n, :] | 1].
    accs = [apool.tile([P, CA], f32, tag=f"acc{j}", name=f"acc{j}")
            for j in range(4)]

    xt_r = xt[:].rearrange("(ko ki) t -> ki ko t", ki=P)

    # HAM warm-up: keep the PE busy during the initial DMAs so the clock
    # gate ramps before real work starts.
    wup = consts.tile([P, HS], f16)
    nc.gpsimd.memset(wup[:], 0.0)
    for _ in range(9):
        warm = ppool.tile([P, 2, HS], f32, tag="lg", name="warm")
        nc.tensor.matmul(warm[:, 0, :], wup[:, 0:P], wup[:], start=True,
                         stop=True)

    def emit_g(w4, xm_sb, oct_i, gi0, pi):
        # reduction matmuls for one subtile of a normalized oct (delayed
        # PD octs so PE always has normalized weights available);
        # interleaved after each subtile's logits so G LDWEIGHTS hide
        # under the longer 512-row logits streams
        gi = gi0 + pi
        rhs = xm_sb[:, oct_i * QS + pi, :]               # [128(tok), 257]
        for j in range(4):
            lhsT = w4[:, pi, j * P:(j + 1) * P]          # [128(tok), 128]
            nc.tensor.matmul(accs[j][:], lhsT, rhs,
                             start=gi == 0, stop=gi == n_tot - 1)

    def finish(e):
        # deferred (2-oct skew) recH expansion + normalize for oct e
        w4, rec = e
        recH = spool.tile([P, QS, S // 2, H], f16, tag="recH")
        nc.scalar.activation(
            out=recH[:],
            in_=rec[:, :, None, :].to_broadcast((P, QS, S // 2, H)),
            func=mybir.ActivationFunctionType.Copy)
        rH = recH[:].rearrange("p t s h -> p t (s h)")
        nc.vector.tensor_mul(out=w4[:, :, 0:HS // 2],
                             in0=w4[:, :, 0:HS // 2], in1=rH)
        nc.vector.tensor_mul(out=w4[:, :, HS // 2:HS],
                             in0=w4[:, :, HS // 2:HS], in1=rH)

    out_sb = opool.tile([P, 4, CA], f32)
    out_r = out[:].rearrange("j p c -> p j c")
    chain = []
    pending = []
    qr = 0
    for blk in range(n_blk):
        x_sb = xpool.tile([P, KO, TT], f16)
        xm_sb = mpool.tile([P, n_sub, CA], f16)
        xm_src = xtm[blk * TT:(blk + 1) * TT, :].rearrange(
            "(sb p) c -> p sb c", p=P)
        if blk == 0:
            # split the first block's DMAs so compute can start after a
            # small head piece arrives; the big e2 constant and the xm
            # tiles (first needed octs later) queue behind the early chunks
            # first x piece via the gpsimd DMA path: its descriptors
            # generate in parallel with wc chunk 0's on the sync engine
            nc.gpsimd.dma_start(x_sb[:, :, 0:256], xt_r[:, :, 0:256])
            nc.sync.dma_start(wc_sb[:, 1, :], wc_r[:, 1, :])
            nc.sync.dma_start(bcr_sb[0:1, :], bcr[:])
            nc.sync.dma_start(x_sb[:, :, 256:512], xt_r[:, :, 256:512])
            # build e2 = exp(bc broadcast) once bcr has landed
            ebc = ppool.tile([P, 2, HS], f32, tag="lg", name="ebc")
            nc.tensor.matmul(ebc[:, 0, :], xpad[0:1, 0:P], bcr_sb[0:1, :],
                             start=True, stop=True)
            nc.scalar.activation(out=e2_sb[:], in_=ebc[:, 0, :],
                                 func=mybir.ActivationFunctionType.Exp)
            for lo, hi in ((512, 1024), (1024, 2048), (2048, 3072),
                           (3072, 4096)):
                nc.sync.dma_start(x_sb[:, :, lo:hi], xt_r[:, :, lo:hi])
        else:
            nc.sync.dma_start(x_sb[:], xt_r[:, :, blk * TT:(blk + 1) * TT])
            nc.sync.dma_start(xm_sb[:], xm_src)
        for oct_i in range(n_sub // QS):
            if blk == 0 and oct_i == 1:
                # deferred first-block xm loads (first needed by G at oct
                # PD+1), issued once the head x/wc pieces have landed
                nc.sync.dma_start(xm_sb[:, 0:n_sub // 2, :],
                                  xm_src[:, 0:n_sub // 2, :])
                nc.sync.dma_start(xm_sb[:, n_sub // 2:, :],
                                  xm_src[:, n_sub // 2:, :])
            if len(chain) > 1:
                finish(chain.pop(0))
            w4 = wpool.tile([P, QS, HS], f16)
            # E-mult subtiles first so the DVE's E-multiply can start
            # early; the bias-matmul subtiles come last.
            for half in range(QS // 2):
                lg2 = ppool.tile([P, 2, HS], f32, tag="lg")
                for si in range(2):
                    sub = half * 2 + si
                    with_bias = sub >= NES
                    gsub = oct_i * QS + sub
                    xk0 = x_sb[:, 0, gsub * P:(gsub + 1) * P]
                    xk1 = x_sb[:, 1, gsub * P:(gsub + 1) * P]
                    if with_bias:
                        nc.tensor.matmul(lg2[:, si, :], xpad[:], bcr_sb[:],
                                         start=True, stop=False)
                    if len(pending) <= PD and qr <= 3:
                        # ramp filler: dependency-free matmuls into the (not
                        # yet started) G accumulators keep the in-order PE
                        # FIFO busy while the softmax chain fills; the first
                        # real G matmul clears them via start=True
                        for wj in range(2 if qr <= 2 else 1):
                            nc.tensor.matmul(accs[(sub + wj) % 4][:],
                                             wup[:, 0:P], wup[:, 0:CA],
                                             start=True, stop=True)
                    nc.tensor.matmul(lg2[:, si, :], xk0, wc_sb[:, 0],
                                     start=not with_bias, stop=False)
                    nc.tensor.matmul(lg2[:, si, :], xk1, wc_sb[:, 1],
                                     start=False, stop=True)
                    if len(pending) > PD:
                        emit_g(*pending[0], sub)
                lo, hi = half * 2, half * 2 + 2
                nc.scalar.activation(out=w4[:, lo:hi, :], in_=lg2[:],
                                     func=mybir.ActivationFunctionType.Exp)
            if NES:
                # in-place E-multiply on the non-bias subtiles
                nc.vector.tensor_mul(
                    out=w4[:, 0:NES, :], in0=w4[:, 0:NES, :],
                    in1=e2_sb[:, None, :].to_broadcast((P, NES, HS)))
            # halving tree over s (contiguous slices in s-major layout)
            # down to den[P, QS, H] in f16, then reciprocal.
            src = w4
            width = HS
            while width > H:
                width //= 2
                dt = f16 if width > H else f32
                v = vpool.tile([P, QS, width], dt, tag=f"v{width}")
                nc.vector.tensor_add(out=v[:], in0=src[:, :, 0:width],
                                     in1=src[:, :, width:2 * width])
                src = v
            # den is strictly positive (sum of exps), so the fast
            # approximate reciprocal (~18 correct bits) is safe here.
            rec = spool.tile([P, QS, H], f32, tag="rec")
            nc.vector.reciprocal_approx_fast(
                rec[:].rearrange("p t h -> p (t h)"),
                src[:].rearrange("p t h -> p (t h)"))
            if len(pending) > PD:
                pending.pop(0)
            chain.append((w4, rec))
            pending.append((w4, xm_sb, oct_i, qr * QS))
            qr += 1
    while chain or pending:
        if chain:
            finish(chain.pop(0))
        if pending:
            e = pending.pop(0)
            if pending:
                for pi in range(QS):
                    emit_g(*e, pi)
            else:
                # warm the SBUF->DRAM path so the real store DMAs don't pay
                # cold-queue startup (regions are overwritten by the real
                # stores of accs 0 and 1)
                nc.sync.dma_start(out[0, 0:P, 0:4], out_sb[:, 0, 0:4])
                nc.gpsimd.dma_start(out[1, 0:P, 0:4], out_sb[:, 1, 0:4])
                # absorb the finish-chain stall ahead of the last oct's G
                for wf in range(8):
                    warmd = ppool.tile([P, 2, HS], f32, tag="lg", name="warmd")
                    nc.tensor.matmul(warmd[:, 0, 0:CA], wup[:, 0:P],
                                     wup[:, 0:CA], start=True, stop=True)
                # final oct: j-major order so acc j's accumulation closes
                # ~0.9us before acc j+1's -> evictions and store DMAs
                # pipeline under the remaining G matmuls
                w4, xm_sb, oct_i, gi0 = e
                for j in range(4):
                    for pi in range(QS):
                        gi = gi0 + pi
                        rhs = xm_sb[:, oct_i * QS + pi, :]
                        lhsT = w4[:, pi, j * P:(j + 1) * P]
                        nc.tensor.matmul(accs[j][:], lhsT, rhs,
                                         start=gi == 0, stop=gi == n_tot - 1)

    # final PSUM evictions: each acc is split across DVE+ACT so it evicts
    # in half the time, and its store DMA is split across the sync and
    # gpsimd descriptor paths so the two half-transfers run in parallel
    for j in range(4):
        nc.vector.tensor_copy(out_sb[:, j, 0:128], accs[j][:, 0:128])
        nc.scalar.activation(out=out_sb[:, j, 128:CA], in_=accs[j][:, 128:CA],
                             func=mybir.ActivationFunctionType.Copy)
        nc.sync.dma_start(out_r[:, j, 0:128], out_sb[:, j, 0:128])
        nc.gpsimd.dma_start(out_r[:, j, 128:CA], out_sb[:, j, 128:CA])


def build_bass(t_tokens=T, finalize=True):
    from contextlib import ExitStack
    nc = bacc.Bacc("TRN2")
    xt = nc.dram_tensor("xt", [C, t_tokens], f16, kind="ExternalInput")
    wc = nc.dram_tensor("wc", [C, HS], f16, kind="ExternalInput")
    bcr = nc.dram_tensor("bcr", [1, HS], f16, kind="ExternalInput")
    xtm = nc.dram_tensor("xtm", [t_tokens, CA], f16, kind="ExternalInput")
    out = nc.dram_tensor("out", [4, P, CA], f32, kind="ExternalOutput")
    with tile.TileContext(nc) as tc:
        with ExitStack() as ctx:
            _emit(ctx, tc, xt, wc, bcr, xtm, out, t_tokens)
    if finalize:
        nc.finalize()
    return nc


def make_device_weights(W_x, b_x, W_slice, b_slice, temperature):
    """Host-side weight fusion (s-major cols: col = s*H + h)."""
    temp = np.clip(np.asarray(temperature, np.float64).reshape(H), 0.5, 5.0)
    Wx3 = np.asarray(W_x, np.float64).reshape(C, H, D)
    Ws = np.asarray(W_slice, np.float64)
    Wc = np.einsum("chd,ds->chs", Wx3, Ws) / temp[None, :, None]
    bc = (np.asarray(b_x, np.float64).reshape(H, D) @ Ws
          + np.asarray(b_slice, np.float64)[None, :]) / temp[:, None]
    wc_dev = np.ascontiguousarray(
        Wc.transpose(0, 2, 1).reshape(C, HS)).astype(ml_dtypes.bfloat16)
    bc_row = bc.T.reshape(HS)                             # [HS] s-major
    bcr_dev = bc_row.astype(ml_dtypes.bfloat16).reshape(1, HS)
    return wc_dev, bcr_dev


def untangle(M):
    """Per-core device output [4, 128, 257] -> G [H, S, C+1] (col C = norm).
    s-major: chunk j row m <-> col q = j*128+m, h = q % 8, s = q // 8."""
    M = np.asarray(M, np.float64).reshape(4 * P, CA)
    return M.reshape(S, H, CA).transpose(1, 0, 2)


def postprocess(core_outs, W_fx, b_fx):
    Wf = np.asarray(W_fx, np.float64).reshape(C, H, D)
    bfx = np.asarray(b_fx, np.float64).reshape(H, D)
    out = np.empty((B, H, S, D), np.float32)
    for b in range(B):
        G = sum(untangle(core_outs[b * SHARDS_PER_B + q]) for q in range(SHARDS_PER_B))
        Mn = G[..., C]                      # [H, S] total softmax mass
        Q = np.einsum("hsc,chd->hsd", G[..., :C], Wf)
        res = (Q + Mn[..., None] * bfx[:, None, :]) / (Mn[..., None] + 0.01)
        out[b] = res.astype(np.float32)
    return out


def make_in_maps(x, wc_dev, bcr_dev):
    x = np.asarray(x)
    in_maps = []
    for core in range(NCORES):
        b, q = core // SHARDS_PER_B, core % SHARDS_PER_B
        xs = x[b, q * T:(q + 1) * T, :]
        xt = np.ascontiguousarray(xs.T.astype(ml_dtypes.bfloat16))
        xtm = np.empty((T, CA), ml_dtypes.bfloat16)
        xtm[:, :C] = xs.astype(ml_dtypes.bfloat16)
        xtm[:, C] = 1.0
        in_maps.append({"xt": xt, "wc": wc_dev, "bcr": bcr_dev, "xtm": xtm})
    return in_maps


_NC_CACHE = {}


def _get_nc():
    if "nc" not in _NC_CACHE:
        _NC_CACHE["nc"] = build_bass()
    return _NC_CACHE["nc"]


def _run(x, W_x, b_x, W_fx, b_fx, W_slice, b_slice, temperature, trace=False):
    wc_dev, bcr_dev = make_device_weights(
        W_x, b_x, W_slice, b_slice, temperature)
    in_maps = make_in_maps(x, wc_dev, bcr_dev)
    res = run_bass_kernel_spmd(_get_nc(), in_maps, core_ids=list(range(NCORES)),
                               trace=trace)
    out = postprocess([r["out"] for r in res.results], W_fx, b_fx)
    return out, res


def kernel(**inputs) -> np.ndarray:
    out, _ = _run(**inputs)
    return out


def kernel_traced(**inputs):
    out, res = _run(**inputs, trace=True)
    return out, res



# revision 33
# speedup vs baseline: 1.0018x; 1.0018x over previous
"""Trainium2 Bass kernel for nn_ContextProjector (moe_routing).

Reference computation:
    projected = split_heads(x @ W_x + b_x)            # (B,H,N,D)
    fx        = split_heads(x @ W_fx + b_fx)          # (B,H,N,D)
    sp        = projected @ W_slice + b_slice         # (B,H,N,S)
    w         = softmax(sp / clip(temp,.5,5))         # (B,H,N,S)
    norm      = w.sum(axis=N)                         # (B,H,S)
    out       = einsum('bhns,bhnd->bhsd', w/(norm+.01), fx)

Key algebraic restructuring (all exact):
  * projected is only used for sp, so fold on host:
        Wc[c,(h,s)] = sum_d W_x[c,(h,d)] W_slice[d,s] / t[h]
    and sp/t = x @ Wc + bc.  The additive bias bc is applied two ways,
    balancing PE against DVE: for NB pairs per oct via the classic
    ones-row matmul into PSUM (PE), for the rest multiplicatively after
    exp (exp(lg+bc) = exp(lg)*E) as a coalesced 2x-mode DVE multiply.
  * fx never exists on device. With w~ the per-token softmax:
        sum_n w~[n,s] (x[n,:] @ W_fx + b_fx)[d]
          = (sum_n w~[n,s] [x[n,:] | 1]) @ [W_fx; b_fx]
    so the device only accumulates G[(h,s), c] = sum_n w~[n,(h,s)] [x|1][n,c]
    into PSUM; the tiny G @ W_fx, the b_fx term, and the final divide by
    (norm+0.01) happen on host in float64. Column c=C of G is the norm.

Column layout is S-MAJOR (col = s*H + h): the softmax denominator then
comes from a log2(S)-deep halving tree of CONTIGUOUS-slice adds (DVE runs
at ~0.5 ns/elem on packed runs vs ~2.3 ns/elem for strided reduces, and
~13 ns per access-pattern row on broadcasts).

All matmul operands are BFLOAT16 (PSUM accumulates fp32): bf16 streams at
the full 2.4 GHz warm PE clock (one rhs column per cycle), whereas fp16
measured ~20% slower; bf16 also converts to the DVE's internal fp32 for
free.  Measured per-op rates (fast clock): N=512 matmul issue gap 216 ns,
N=257 gap 110 ns (LDWEIGHTS fully hidden), DVE TT bf16 2x-mode
0.52 ns/elem + ~162 ns/op, ACT 1x dtype-independent (FD+352)/1.2 ns.

Engine assignment per OCT (8 subtiles = 1024 tokens; fixed costs amortized):
  PE : per pair: lg2 psum = 2 K-chunk matmuls per subtile (+ bias matmul
       for the NBS=1 last subtile); per oct: 32 G matmuls (4 accs x 8
       subtiles), ap 257.  PE is the bottleneck engine (~93% occupancy,
       near the 78.6 TF/s roofline for the 8.6 GF of work per core).
  ACT: 4x exp [128,2,512] PSUM->SBUF per oct; recH = half-width broadcast
       expansion of rec, Copy activation
  DVE: in-place E-multiply on the non-bias subtiles (exp(lg+bc)=exp(lg)*E);
       halving tree to den (bf16, last level f32);
       reciprocal_approx_fast (f32, ~5x cheaper than reciprocal);
       2x coalesced normalize (two halves vs recH)
  GpS: only head/tail DMA descriptor generation (its compute ops contend
       for the DVE's shared SBUF port)
The recH expansion and normalize for oct q are deferred to oct q+2's
emission slot, which breaks the exp->E->tree->recip->recH->exp serial
cycle across octs.  G matmuls run PD=4 octs behind their producers (the
deep pending queue keeps the PE busy over the tail finish-chains).
Ramp/tail PE idle is filled with dependency-free matmuls into the
not-yet-started G accumulators (the first real G matmul clears them via
start=True).  Constants bc/e2 are built on device from a single [1,HS]
row (K=1 ones matmul broadcast + exp) instead of replicated DMAs; the
head x/wc DMAs are split so first compute starts ~2.5 us after the NEFF
preamble; the final oct emits G j-major so the 4 accumulator stores
pipeline; store DMA queues are pre-warmed.
"""

import numpy as np
import ml_dtypes

import concourse.bass as bass
import concourse.mybir as mybir
import concourse.tile as tile
from concourse import bacc
from concourse.bass_utils import run_bass_kernel_spmd

# Problem shape (hardcoded per contract)
B, N, C = 2, 65536, 256
H, D, S = 8, 64, 64
HS = H * S    # 512
P = 128
NCORES = 8
SHARDS_PER_B = NCORES // B   # 4
T = N // SHARDS_PER_B        # 16384 tokens per core
CA = C + 1                   # token-major x augmented with a ones column

PD = 4     # G-matmul pipeline depth (octs)
TT = 2048  # tokens per DMA block
QT = 1024  # tokens per oct (chain work unit)
QS = QT // P                 # 8 subtiles per oct
NBS = 1    # subtiles per oct whose bias rides the PE (rest use DVE E-mult)
NES = QS - NBS               # subtiles per oct using the DVE E-multiply

f16 = mybir.dt.bfloat16
f32 = mybir.dt.float32


def _emit(ctx, tc, xt, wc, bcr, xtm, out, t_tokens):
    nc = tc.nc
    KO = C // P              # 2 K-chunks of x
    n_blk = t_tokens // TT
    n_sub = TT // P          # subtiles (128 tokens) per block
    n_tot = t_tokens // P    # total subtiles (for G start/stop flags)

    consts = ctx.enter_context(tc.tile_pool(name="consts", bufs=1))
    xpool = ctx.enter_context(tc.tile_pool(name="xpool", bufs=4))
    mpool = ctx.enter_context(tc.tile_pool(name="mpool", bufs=5))
    wpool = ctx.enter_context(tc.tile_pool(name="wpool", bufs=PD + 4))
    mpool_extra = None  # xm lifetime covered by mpool bufs below
    vpool = ctx.enter_context(tc.tile_pool(name="vpool", bufs=3))
    spool = ctx.enter_context(tc.tile_pool(name="spool", bufs=4))
    ppool = ctx.enter_context(tc.tile_pool(name="ppool", bufs=2, space="PSUM"))
    apool = ctx.enter_context(tc.tile_pool(name="apool", bufs=1, space="PSUM"))
    opool = ctx.enter_context(tc.tile_pool(name="opool", bufs=1))

    # Constant weights, resident in SBUF for the whole kernel.  Split the
    # wc DMA per K-chunk so the first logits matmul only waits on chunk 0.
    wc_r = wc[:].rearrange("(ko ki) n -> ki ko n", ki=P)
    wc_sb = consts.tile([P, KO, HS], f16)
    nc.sync.dma_start(wc_sb[:, 0, :], wc_r[:, 0, :])
    # bcr: only row 0 (= bc) is real; rows 1-127 are zeroed so the
    # ones-row bias matmul multiplies them by zero safely.
    bcr_sb = consts.tile([P, HS], f16)
    nc.vector.memset(bcr_sb[:], 0.0)
    # e2 = exp(bc) broadcast to all partitions, built on device: a K=1
    # ones-column matmul replicates bc across partitions, then one exp.
    e2_sb = consts.tile([P, HS], f16)
    # Bias K-chunk lhsT: row 0 ones, rest zero -> adds bcr row 0 once.
    xpad = consts.tile([P, P], f16)
    nc.vector.memset(xpad[:], 0.0)
    nc.vector.memset(xpad[0:1, :], 1.0)

    # Persistent PSUM accumulators: chunk j holds
    # G[cols 128j..128j+128, 257] = sum_n w~[n, col] * [x[n, :] | 1].
    accs = [apool.tile([P, CA], f32, tag=f"acc{j}", name=f"acc{j}")
            for j in range(4)]

    xt_r = xt[:].rearrange("(ko ki) t -> ki ko t", ki=P)

    # HAM warm-up: keep the PE busy during the initial DMAs so the clock
    # gate ramps before real work starts.
    wup = consts.tile([P, HS], f16)
    nc.gpsimd.memset(wup[:], 0.0)
    for _ in range(9):
        warm = ppool.tile([P, 2, HS], f32, tag="lg", name="warm")
        nc.tensor.matmul(warm[:, 0, :], wup[:, 0:P], wup[:], start=True,
                         stop=True)

    def emit_g(w4, xm_sb, oct_i, gi0, pi):
        # reduction matmuls for one subtile of a normalized oct (delayed
        # PD octs so PE always has normalized weights available);
        # interleaved after each subtile's logits so G LDWEIGHTS hide
        # under the longer 512-row logits streams
        gi = gi0 + pi
        rhs = xm_sb[:, oct_i * QS + pi, :]               # [128(tok), 257]
        for j in range(4):
            lhsT = w4[:, pi, j * P:(j + 1) * P]          # [128(tok), 128]
            nc.tensor.matmul(accs[j][:], lhsT, rhs,
                             start=gi == 0, stop=gi == n_tot - 1)

    def finish(e):
        # deferred (2-oct skew) recH expansion + normalize for oct e
        w4, rec = e
        recH = spool.tile([P, QS, S // 2, H], f16, tag="recH")
        nc.scalar.activation(
            out=recH[:],
            in_=rec[:, :, None, :].to_broadcast((P, QS, S // 2, H)),
            func=mybir.ActivationFunctionType.Copy)
        rH = recH[:].rearrange("p t s h -> p t (s h)")
        nc.vector.tensor_mul(out=w4[:, :, 0:HS // 2],
                             in0=w4[:, :, 0:HS // 2], in1=rH)
        nc.vector.tensor_mul(out=w4[:, :, HS // 2:HS],
                             in0=w4[:, :, HS // 2:HS], in1=rH)

    out_sb = opool.tile([P, 4, CA], f32)
    out_r = out[:].rearrange("j p c -> p j c")
    chain = []
    pending = []
    qr = 0
    for blk in range(n_blk):
        x_sb = xpool.tile([P, KO, TT], f16)
        xm_sb = mpool.tile([P, n_sub, CA], f16)
        xm_src = xtm[blk * TT:(blk + 1) * TT, :].rearrange(
            "(sb p) c -> p sb c", p=P)
        if blk == 0:
            # split the first block's DMAs so compute can start after a
            # small head piece arrives; the big e2 constant and the xm
            # tiles (first needed octs later) queue behind the early chunks
            # first x piece via the gpsimd DMA path: its descriptors
            # generate in parallel with wc chunk 0's on the sync engine
            nc.gpsimd.dma_start(x_sb[:, :, 0:256], xt_r[:, :, 0:256])
            nc.sync.dma_start(wc_sb[:, 1, :], wc_r[:, 1, :])
            nc.sync.dma_start(bcr_sb[0:1, :], bcr[:])
            nc.sync.dma_start(x_sb[:, :, 256:512], xt_r[:, :, 256:512])
            # build e2 = exp(bc broadcast) once bcr has landed
            ebc = ppool.tile([P, 2, HS], f32, tag="lg", name="ebc")
            nc.tensor.matmul(ebc[:, 0, :], xpad[0:1, 0:P], bcr_sb[0:1, :],
                             start=True, stop=True)
            nc.scalar.activation(out=e2_sb[:], in_=ebc[:, 0, :],
                                 func=mybir.ActivationFunctionType.Exp)
            for i in range(1, 4):
                sl = slice(i * TT // 4, (i + 1) * TT // 4)
                nc.sync.dma_start(x_sb[:, :, sl], xt_r[:, :, sl])
        else:
            nc.sync.dma_start(x_sb[:], xt_r[:, :, blk * TT:(blk + 1) * TT])
            nc.sync.dma_start(xm_sb[:], xm_src)
        for oct_i in range(n_sub // QS):
            if blk == 0 and oct_i == 1:
                # deferred first-block xm loads (first needed by G at oct
                # PD+1), issued once the head x/wc pieces have landed
                nc.sync.dma_start(xm_sb[:, 0:n_sub // 2, :],
                                  xm_src[:, 0:n_sub // 2, :])
                nc.sync.dma_start(xm_sb[:, n_sub // 2:, :],
                                  xm_src[:, n_sub // 2:, :])
            if len(chain) > 1:
                finish(chain.pop(0))
            w4 = wpool.tile([P, QS, HS], f16)
            # E-mult subtiles first so the DVE's E-multiply can start
            # early; the bias-matmul subtiles come last.
            for half in range(QS // 2):
                lg2 = ppool.tile([P, 2, HS], f32, tag="lg")
                for si in range(2):
                    sub = half * 2 + si
                    with_bias = sub >= NES
                    gsub = oct_i * QS + sub
                    xk0 = x_sb[:, 0, gsub * P:(gsub + 1) * P]
                    xk1 = x_sb[:, 1, gsub * P:(gsub + 1) * P]
                    if with_bias:
                        nc.tensor.matmul(lg2[:, si, :], xpad[:], bcr_sb[:],
                                         start=True, stop=False)
                    if len(pending) <= PD and qr <= 3:
                        # ramp filler: dependency-free matmuls into the (not
                        # yet started) G accumulators keep the in-order PE
                        # FIFO busy while the softmax chain fills; the first
                        # real G matmul clears them via start=True
                        for wj in range(2 if qr <= 2 else 1):
                            nc.tensor.matmul(accs[(sub + wj) % 4][:],
                                             wup[:, 0:P], wup[:, 0:CA],
                                             start=True, stop=True)
                    nc.tensor.matmul(lg2[:, si, :], xk0, wc_sb[:, 0],
                                     start=not with_bias, stop=False)
                    nc.tensor.matmul(lg2[:, si, :], xk1, wc_sb[:, 1],
                                     start=False, stop=True)
                    if len(pending) > PD:
                        emit_g(*pending[0], sub)
                lo, hi = half * 2, half * 2 + 2
                nc.scalar.activation(out=w4[:, lo:hi, :], in_=lg2[:],
                                     func=mybir.ActivationFunctionType.Exp)
            if NES:
                # in-place E-multiply on the non-bias subtiles
                nc.vector.tensor_mul(
                    out=w4[:, 0:NES, :], in0=w4[:, 0:NES, :],
                    in1=e2_sb[:, None, :].to_broadcast((P, NES, HS)))
            # halving tree over s (contiguous slices in s-major layout)
            # down to den[P, QS, H] in f16, then reciprocal.
            src = w4
            width = HS
            while width > H:
                width //= 2
                dt = f16 if width > H else f32
                v = vpool.tile([P, QS, width], dt, tag=f"v{width}")
                nc.vector.tensor_add(out=v[:], in0=src[:, :, 0:width],
                                     in1=src[:, :, width:2 * width])
                src = v
            # den is strictly positive (sum of exps), so the fast
            # approximate reciprocal (~18 correct bits) is safe here.
            rec = spool.tile([P, QS, H], f32, tag="rec")
            nc.vector.reciprocal_approx_fast(
                rec[:].rearrange("p t h -> p (t h)"),
                src[:].rearrange("p t h -> p (t h)"))
            if len(pending) > PD:
                pending.pop(0)
            chain.append((w4, rec))
            pending.append((w4, xm_sb, oct_i, qr * QS))
            qr += 1
    while chain or pending:
        if chain:
            finish(chain.pop(0))
        if pending:
            e = pending.pop(0)
            if pending:
                for pi in range(QS):
                    emit_g(*e, pi)
            else:
                # warm the SBUF->DRAM path so the real store DMAs don't pay
                # cold-queue startup (regions are overwritten by the real
                # stores of accs 0 and 1)
                nc.sync.dma_start(out[0, 0:P, 0:4], out_sb[:, 0, 0:4])
                nc.gpsimd.dma_start(out[1, 0:P, 0:4], out_sb[:, 1, 0:4])
                # absorb the finish-chain stall ahead of the last oct's G
                for wf in range(8):
                    warmd = ppool.tile([P, 2, HS], f32, tag="lg", name="warmd")
                    nc.tensor.matmul(warmd[:, 0, 0:CA], wup[:, 0:P],
                                     wup[:, 0:CA], start=True, stop=True)
                # final oct: j-major order so acc j's accumulation closes
                # ~0.9us before acc j+1's -> evictions and store DMAs
                # pipeline under the remaining G matmuls
                w4, xm_sb, oct_i, gi0 = e
                for j in range(4):
                    for pi in range(QS):
                        gi = gi0 + pi
                        rhs = xm_sb[:, oct_i * QS + pi, :]
                        lhsT = w4[:, pi, j * P:(j + 1) * P]
                        nc.tensor.matmul(accs[j][:], lhsT, rhs,
                                         start=gi == 0, stop=gi == n_tot - 1)

    # final PSUM evictions: each acc is split across DVE+ACT so it evicts
    # in half the time, and its store DMA is split across the sync and
    # gpsimd descriptor paths so the two half-transfers run in parallel
    for j in range(4):
        nc.vector.tensor_copy(out_sb[:, j, 0:128], accs[j][:, 0:128])
        nc.scalar.activation(out=out_sb[:, j, 128:CA], in_=accs[j][:, 128:CA],
                             func=mybir.ActivationFunctionType.Copy)
        nc.sync.dma_start(out_r[:, j, 0:128], out_sb[:, j, 0:128])
        nc.gpsimd.dma_start(out_r[:, j, 128:CA], out_sb[:, j, 128:CA])


def build_bass(t_tokens=T, finalize=True):
    from contextlib import ExitStack
    nc = bacc.Bacc("TRN2")
    xt = nc.dram_tensor("xt", [C, t_tokens], f16, kind="ExternalInput")
    wc = nc.dram_tensor("wc", [C, HS], f16, kind="ExternalInput")
    bcr = nc.dram_tensor("bcr", [1, HS], f16, kind="ExternalInput")
    xtm = nc.dram_tensor("xtm", [t_tokens, CA], f16, kind="ExternalInput")
    out = nc.dram_tensor("out", [4, P, CA], f32, kind="ExternalOutput")
    with tile.TileContext(nc) as tc:
        with ExitStack() as ctx:
            _emit(ctx, tc, xt, wc, bcr, xtm, out, t_tokens)
    if finalize:
        nc.finalize()
    return nc


def make_device_weights(W_x, b_x, W_slice, b_slice, temperature):
    """Host-side weight fusion (s-major cols: col = s*H + h)."""
    temp = np.clip(np.asarray(temperature, np.float64).reshape(H), 0.5, 5.0)
    Wx3 = np.asarray(W_x, np.float64).reshape(C, H, D)
    Ws = np.asarray(W_slice, np.float64)
    Wc = np.einsum("chd,ds->chs", Wx3, Ws) / temp[None, :, None]
    bc = (np.asarray(b_x, np.float64).reshape(H, D) @ Ws
          + np.asarray(b_slice, np.float64)[None, :]) / temp[:, None]
    wc_dev = np.ascontiguousarray(
        Wc.transpose(0, 2, 1).reshape(C, HS)).astype(ml_dtypes.bfloat16)
    bc_row = bc.T.reshape(HS)                             # [HS] s-major
    bcr_dev = bc_row.astype(ml_dtypes.bfloat16).reshape(1, HS)
    return wc_dev, bcr_dev


def untangle(M):
    """Per-core device output [4, 128, 257] -> G [H, S, C+1] (col C = norm).
    s-major: chunk j row m <-> col q = j*128+m, h = q % 8, s = q // 8."""
    M = np.asarray(M, np.float64).reshape(4 * P, CA)
    return M.reshape(S, H, CA).transpose(1, 0, 2)


def postprocess(core_outs, W_fx, b_fx):
    Wf = np.asarray(W_fx, np.float64).reshape(C, H, D)
    bfx = np.asarray(b_fx, np.float64).reshape(H, D)
    out = np.empty((B, H, S, D), np.float32)
    for b in range(B):
        G = sum(untangle(core_outs[b * SHARDS_PER_B + q]) for q in range(SHARDS_PER_B))
        Mn = G[..., C]                      # [H, S] total softmax mass
        Q = np.einsum("hsc,chd->hsd", G[..., :C], Wf)
        res = (Q + Mn[..., None] * bfx[:, None, :]) / (Mn[..., None] + 0.01)
        out[b] = res.astype(np.float32)
    return out


def make_in_maps(x, wc_dev, bcr_dev):
    x = np.asarray(x)
    in_maps = []
    for core in range(NCORES):
        b, q = core // SHARDS_PER_B, core % SHARDS_PER_B
        xs = x[b, q * T:(q + 1) * T, :]
        xt = np.ascontiguousarray(xs.T.astype(ml_dtypes.bfloat16))
        xtm = np.empty((T, CA), ml_dtypes.bfloat16)
        xtm[:, :C] = xs.astype(ml_dtypes.bfloat16)
        xtm[:, C] = 1.0
        in_maps.append({"xt": xt, "wc": wc_dev, "bcr": bcr_dev, "xtm": xtm})
    return in_maps


_NC_CACHE = {}


def _get_nc():
    if "nc" not in _NC_CACHE:
        _NC_CACHE["nc"] = build_bass()
    return _NC_CACHE["nc"]


def _run(x, W_x, b_x, W_fx, b_fx, W_slice, b_slice, temperature, trace=False):
    wc_dev, bcr_dev = make_device_weights(
        W_x, b_x, W_slice, b_slice, temperature)
    in_maps = make_in_maps(x, wc_dev, bcr_dev)
    res = run_bass_kernel_spmd(_get_nc(), in_maps, core_ids=list(range(NCORES)),
                               trace=trace)
    out = postprocess([r["out"] for r in res.results], W_fx, b_fx)
    return out, res


def kernel(**inputs) -> np.ndarray:
    out, _ = _run(**inputs)
    return out


def kernel_traced(**inputs):
    out, res = _run(**inputs, trace=True)
    return out, res

